# revision 8
# baseline (speedup 1.0000x reference)
"""Grouped per-adapter LoRA kernel for Trainium2 (8 NeuronCores).

Strategy: shard BY ADAPTER. Core a receives the tokens routed to adapter a
(gathered + transposed on host), plus only that adapter's A/B weight tables
(rank-masked on host, which is exactly equivalent to the reference's
rank-masking of the intermediate activations). Each core then runs a dense
two-stage GEMM entirely from SBUF-resident weights:

    y[r, t]   = sum_k A[k, r] * xT[k, t]      (down-projection, PSUM accum)
    out[t, o] = sum_r y[r, t] * B[r, o]       (up-projection)

All matmul operands are fp16 (exact products, fp32 PSUM accumulation); the
output is written fp16 and widened on host. Module pairing is (m0, m2) so
m2's tables live on SBUF partitions 64-127 and the two up-projections run on
disjoint PE row groups concurrently with no B-table duplication in HBM.

Engine assignment (the previous revision stalled because x-block DMAs were
issued by the scalar engine, queued behind PSUM->SBUF copies):
  - sync HWDGE:     all x-block DMAs issued up-front (they are never gated,
                    so no head-of-line blocking), then the B table, then the
                    per-strip 3 MB output DMAs.
  - scalar HWDGE:   A-table chunks, then half the PSUM->SBUF out-copies.
  - vector:         y copies + other half of the out-copies.
Output DRAM layout is [C, M*O] == the final output layout, so each strip is
one fully-contiguous DMA and host unshard is a plain row scatter.
"""

import sys

if "/opt/trn_rl_repo" not in sys.path:
    sys.path.insert(0, "/opt/trn_rl_repo")

import numpy as np

N_CORES = 8
P = 128   # partition width
NBLK = 256  # main token-block size
KG_A = 4  # A-table DMA chunks

_prog_cache: dict = {}
last_run_results = None  # BassKernelResults of the most recent dispatch
last_ctx = None          # (nc, in_maps) of the most recent dispatch


def _choose_capacity(nmax: int) -> int:
    """Per-core token capacity: smallest multiple of 64 >= nmax."""
    return ((max(nmax, 1) + 63) // 64) * 64


def _block_list(C: int) -> tuple:
    """Token blocks of NBLK plus one smaller tail (multiple of 64), tail
    LAST so the exposed pipeline drain is small."""
    n, rem = divmod(C, NBLK)
    blocks = [NBLK] * n + ([rem] if rem else [])
    return tuple(blocks)


def _block_kg(bi: int, nb: int) -> int:
    """x DMA chunks per block: fine-grained for block 0 (pipeline fill),
    coarse afterwards (DMA efficiency)."""
    if nb < NBLK:
        return 1
    return 4 if bi == 0 else 1


def _build_program(C: int, H: int, M: int, R: int, O: int):
    """Trace + compile the single SPMD program (shared by all 8 cores)."""
    import concourse.bass as bass
    import concourse.mybir as mybir
    import concourse.tile as tile
    from concourse import bacc

    f32 = mybir.dt.float32
    f16 = mybir.dt.float16
    KT = H // P          # contraction tiles
    KS_A = KT // KG_A
    J = O // 512         # up-projection PSUM tiles per module
    MO = M * O
    blocks = _block_list(C)

    nc = bacc.Bacc("TRN2", target_bir_lowering=False, debug=False,
                   num_devices=N_CORES)

    # xh is flat; block b with KG chunks holds [KG, P, KS, nb] where
    # xh[g, p, k, n] = xT[(g*KS + k)*P + p, t0 + n].
    xh = nc.dram_tensor("xh", [C * H], f16, kind="ExternalInput")
    # wa[g, p, k, j, r] = A_masked[mod(j), (g*KS_A + k)*P + p, r] with
    # module order mod = (0, 2, 1)  (j=0 -> m0, j=1 -> m2, j=2 -> m1).
    wa = nc.dram_tensor("wa", [KG_A, P, KS_A, M, R], f16, kind="ExternalInput")
    # wb[r, m, o] = B[m, r, o] (natural module order, no duplication).
    wb = nc.dram_tensor("wb", [R, M, O], f16, kind="ExternalInput")
    # fp16 output in the FINAL layout [C, M*O]; host widens + row-scatters.
    out = nc.dram_tensor("out", [C, MO], f16, kind="ExternalOutput")

    with tile.TileContext(nc) as tc:
        with (
            tc.tile_pool(name="wgt", bufs=1) as wpool,
            tc.tile_pool(name="yts", bufs=2) as ypool,
            tc.tile_pool(name="ost", bufs=3) as opool,
            tc.tile_pool(name="py", bufs=1, space=bass.MemorySpace.PSUM) as pyp,
            tc.tile_pool(name="pu", bufs=3, space=bass.MemorySpace.PSUM) as pup,
        ):
            wa_t = wpool.tile([P, KT, M, R], f16)
            # wb_t rows 0:R = (m0, m1) tables, rows R:2R col 0 = m2's table.
            wb_t = wpool.tile([2 * R, 2, O], f16)
            for g in range(KG_A):
                nc.scalar.dma_start(wa_t[:, g * KS_A:(g + 1) * KS_A, :, :],
                                    wa[g])

            # One SBUF tile per block (never recycled -> the up-front x DMAs
            # are never gated on compute, so they can't head-of-line block
            # the sync ring).
            xb_tiles = []
            t0 = 0
            for bi, nb in enumerate(blocks):
                xb = wpool.tile([P, KT, nb], f16, tag=f"xb{bi}", name=f"xb{bi}")
                kg = _block_kg(bi, nb)
                ks = KT // kg
                for g in range(kg):
                    off = t0 * H + g * (P * ks * nb)
                    xv = xh[off:off + P * ks * nb].rearrange(
                        "(p k n) -> p k n", p=P, k=ks, n=nb
                    )
                    nc.sync.dma_start(xb[:, g * ks:(g + 1) * ks, :], xv)
                xb_tiles.append(xb)
                t0 += nb
                if bi == 1:
                    # B tables ride between x blocks 1 and 2: early enough
                    # that block 0's up-projection (the first strip work,
                    # ~15us in) is never gated on them, late enough not to
                    # delay block 0/1 down-projections.
                    nc.sync.dma_start(wb_t[0:R, 0:2, :], wb[:, 0:2, :])
                    nc.sync.dma_start(wb_t[R:2 * R, 0, :], wb[:, 2, :])
            if len(blocks) < 2:
                nc.sync.dma_start(wb_t[0:R, 0:2, :], wb[:, 0:2, :])
                nc.sync.dma_start(wb_t[R:2 * R, 0, :], wb[:, 2, :])

            # PE warm-up: junk matmuls keep the HAM activity window busy
            # while the first x chunks stream in, so the PE clock gate is
            # at 8/8 when real work arrives.
            wtile = wpool.tile([P, P], f16)
            nc.gpsimd.memset(wtile[:], 0.0)
            for _ in range(24):
                wu = pup.tile([P, P], f32, tag="ou")
                nc.tensor.matmul(wu[:], wtile[:], wtile[:], start=True, stop=True)

            cp = 0   # PSUM->SBUF copy counter (vector/scalar balancing)

            def _route_copy(dst, src_):
                nonlocal cp
                if cp % 2 == 1:
                    nc.scalar.copy(dst, src_)
                else:
                    nc.vector.tensor_copy(dst, src_)
                cp += 1

            def emit_part(bt0, bnb, byts02, byts1, os_map, s0, which):
                """Up-projection for one 128-row strip: which=0 -> the fused
                m0/m2 pair (concurrent PE row groups), which=1 -> lone m1.
                PSUM tiles are 1024 wide (2 matmuls each, one bank per
                matmul) so each PSUM->SBUF copy moves 2x the elements --
                halves the per-copy fixed cost. Each part ships its own
                column ranges as soon as its copies land."""
                sl = min(P, bnb - s0)
                os_ = os_map[s0]
                r0, r1 = bt0 + s0, bt0 + s0 + sl
                if which == 0:
                    for j in range(J // 2):
                        c0 = j * 1024
                        ou0 = pup.tile([P, 1024], f32, tag="ou")
                        ou2 = pup.tile([P, 1024], f32, tag="ou")
                        for h in (0, 1):
                            nc.tensor.matmul(
                                ou0[:sl, h * 512:(h + 1) * 512],
                                byts02[0:R, s0:s0 + sl],
                                wb_t[0:R, 0, c0 + h * 512:c0 + (h + 1) * 512],
                                start=True, stop=True,
                            )
                            nc.tensor.matmul(
                                ou2[:sl, h * 512:(h + 1) * 512],
                                byts02[R:2 * R, s0:s0 + sl],
                                wb_t[R:2 * R, 0, c0 + h * 512:c0 + (h + 1) * 512],
                                start=True, stop=True,
                            )
                        _route_copy(os_[:sl, c0:c0 + 1024], ou0[:sl, :])
                        _route_copy(os_[:sl, 2 * O + c0:2 * O + c0 + 1024],
                                    ou2[:sl, :])
                    # ship m0+m2 columns now: [sl, 2, O] strided view of the
                    # [C, M*O] output (col ranges 0:O and 2O:3O).
                    nc.sync.dma_start(
                        out[r0:r1].rearrange("t (c o) -> t c o", c=M, o=O)[:, 0:M:2, :],
                        os_[:sl].rearrange("t (c o) -> t c o", c=M, o=O)[:, 0:M:2, :],
                    )
                else:
                    for j in range(J // 2):
                        c0 = j * 1024
                        ou1 = pup.tile([P, 1024], f32, tag="ou")
                        for h in (0, 1):
                            nc.tensor.matmul(
                                ou1[:sl, h * 512:(h + 1) * 512],
                                byts1[0:R, s0:s0 + sl],
                                wb_t[0:R, 1, c0 + h * 512:c0 + (h + 1) * 512],
                                start=True, stop=True,
                            )
                        _route_copy(os_[:sl, O + c0:O + c0 + 1024], ou1[:sl, :])
                    nc.sync.dma_start(out[r0:r1, O:2 * O], os_[:sl, O:2 * O])

            # Software pipeline with a one-block lag: block b's up-projection
            # parts are emitted BETWEEN block b+1's down-projection chains.
            pend = None  # (t0, nb, yts02, yts1, os_map, parts)
            t0 = 0
            for bi, nb in enumerate(blocks):
                last = bi == len(blocks) - 1
                xb = xb_tiles[bi]
                yts02 = ypool.tile([2 * R, nb], f16, tag="yt02")
                yts1 = ypool.tile([R, nb], f16, tag="yt1")
                os_map = {s0: opool.tile([P, MO], f16, tag="os", name=f"os{bi}")
                          for s0 in range(0, nb, P)}
                parts = [(s0, w) for s0 in range(0, nb, P) for w in (0, 1)]
                ppart = pend[5] if pend else []
                half = (len(ppart) + 1) // 2

                # ---- chain A: modules (m0, m2) fused ----
                y02 = pyp.tile([2 * R, nb], f32, tag="y02")
                for k in range(KT):
                    nc.tensor.matmul(
                        y02[:], wa_t[:, k, 0:2, :], xb[:, k, :],
                        start=(k == 0), stop=(k == KT - 1),
                    )
                nc.vector.tensor_copy(yts02[:], y02[:])
                for s0_, w_ in ppart[:half]:
                    emit_part(pend[0], pend[1], pend[2], pend[3], pend[4],
                              s0_, w_)

                # ---- chain B: module m1 ----
                y1 = pyp.tile([R, nb], f32, tag="y1")
                for k in range(KT):
                    nc.tensor.matmul(
                        y1[:], wa_t[:, k, 2, :], xb[:, k, :],
                        start=(k == 0), stop=(k == KT - 1),
                    )
                nc.vector.tensor_copy(yts1[:], y1[:])
                for s0_, w_ in ppart[half:]:
                    emit_part(pend[0], pend[1], pend[2], pend[3], pend[4],
                              s0_, w_)

                if last:
                    for s0_, w_ in parts:
                        emit_part(t0, nb, yts02, yts1, os_map, s0_, w_)

                pend = (t0, nb, yts02, yts1, os_map, parts)
                t0 += nb

    nc.compile()
    return nc


def _get_program(C: int, H: int, M: int, R: int, O: int):
    key = (C, H, M, R, O)
    if key not in _prog_cache:
        _prog_cache[key] = _build_program(C, H, M, R, O)
    return _prog_cache[key]


def _ensure_profile_hook_module():
    """bass_utils imports antenv.axon_hooks when BASS_TRACE is set; this
    container's antenv package lacks that module. Register a stub returning
    no hook (bass_utils then skips tracing gracefully) unless something
    already provided a real one."""
    import types
    try:
        import antenv.axon_hooks  # noqa: F401
    except ImportError:
        if "antenv.axon_hooks" not in sys.modules:
            mod = types.ModuleType("antenv.axon_hooks")
            mod.get_axon_ntff_profile_hook = lambda: None
            sys.modules["antenv.axon_hooks"] = mod


def kernel(x, lora_a, lora_b, token_adapter_ids, adapter_ranks):
    from concourse.bass_utils import run_bass_kernel_spmd

    _ensure_profile_hook_module()

    x = np.ascontiguousarray(np.asarray(x, dtype=np.float32))
    la = np.array(np.asarray(lora_a), dtype=np.float32, copy=True)  # [M,A,H,R]
    lb = np.ascontiguousarray(np.asarray(lora_b), dtype=np.float32)  # [M,A,R,O]
    ids = np.asarray(token_adapter_ids).astype(np.int64)
    ranks = np.asarray(adapter_ranks).astype(np.int64)

    T, H = x.shape
    M, A, _, R = la.shape
    O = lb.shape[-1]
    assert A <= N_CORES, "one adapter per core"
    assert H % P == 0 and O % 512 == 0

    # Rank masking: zeroing A's columns >= rank_a makes the corresponding
    # intermediate columns exactly 0.0, which is bit-identical to the
    # reference masking the intermediate itself.
    for a in range(A):
        la[:, a, :, int(ranks[a]):] = 0.0

    perms = [np.nonzero(ids == a)[0] for a in range(A)]
    nmax = max(pp.size for pp in perms)
    C = _choose_capacity(nmax)
    blocks = _block_list(C)

    nc = _get_program(C, H, M, R, O)

    KT = H // P
    KS_A = KT // KG_A
    mod_order = (0, 2, 1)
    in_maps = []
    for a in range(N_CORES):
        if a < A:
            perm = perms[a]
            xg = np.zeros((C, H), np.float16)
            xg[:perm.size] = x[perm]  # fp32 -> fp16
            # flat per-block chunked layout [KG, P, KS, nb]
            xh = np.empty(C * H, np.float16)
            t0 = 0
            for bi, nb in enumerate(blocks):
                kg = _block_kg(bi, nb)
                ks = KT // kg
                seg = xg[t0:t0 + nb]  # [nb, H]
                xh[t0 * H:(t0 + nb) * H] = (
                    seg.reshape(nb, kg, ks, P).transpose(1, 3, 2, 0).reshape(-1)
                )
                t0 += nb
            am = la[:, a][list(mod_order)]  # [M, H, R] in (m0, m2, m1) order
            wa_h = np.ascontiguousarray(
                am.reshape(M, KG_A, KS_A, P, R).transpose(1, 3, 2, 0, 4)
            ).astype(np.float16)
            wb_h = np.ascontiguousarray(
                lb[:, a].transpose(1, 0, 2)
            ).astype(np.float16)  # [R, M, O]
        else:
            xh = np.zeros(C * H, np.float16)
            wa_h = np.zeros((KG_A, P, KS_A, M, R), np.float16)
            wb_h = np.zeros((R, M, O), np.float16)
        in_maps.append({"xh": xh, "wa": wa_h, "wb": wb_h})

    global last_run_results, last_ctx
    last_ctx = (nc, in_maps)
    last_run_results = run_bass_kernel_spmd(nc, in_maps, list(range(N_CORES)))
    res = last_run_results.results

    out_full = np.empty((T, M * O), np.float32)
    for a in range(A):
        perm = perms[a]
        if perm.size == 0:
            continue
        r = res[a]["out"]  # [C, M*O] fp16
        out_full[perm] = r[:perm.size, :]
    return out_full


# revision 10
# speedup vs baseline: 1.0652x; 1.0652x over previous
"""Grouped per-adapter LoRA kernel for Trainium2 (8 NeuronCores).

Strategy: shard BY ADAPTER. Core a receives the tokens routed to adapter a
(gathered + transposed on host), plus only that adapter's A/B weight tables
(rank-masked on host, which is exactly equivalent to the reference's
rank-masking of the intermediate activations). Each core then runs a dense
two-stage GEMM entirely from SBUF-resident weights:

    y[r, t]   = sum_k A[k, r] * xT[k, t]      (down-projection, PSUM accum)
    out[t, o] = sum_r y[r, t] * B[r, o]       (up-projection)

All matmul operands are fp16 (exact products, fp32 PSUM accumulation); the
output is written fp16 and widened on host. Module pairing is (m0, m2) so
m2's tables live on SBUF partitions 64-127 and the two up-projections run on
disjoint PE row groups concurrently with no B-table duplication in HBM.

Engine assignment (the previous revision stalled because x-block DMAs were
issued by the scalar engine, queued behind PSUM->SBUF copies):
  - sync HWDGE:     all x-block DMAs issued up-front (they are never gated,
                    so no head-of-line blocking), then the B table, then the
                    per-strip 3 MB output DMAs.
  - scalar HWDGE:   A-table chunks, then half the PSUM->SBUF out-copies.
  - vector:         y copies + other half of the out-copies.
Output DRAM layout is [C, M*O] == the final output layout, so each strip is
one fully-contiguous DMA and host unshard is a plain row scatter.
"""

import sys

if "/opt/trn_rl_repo" not in sys.path:
    sys.path.insert(0, "/opt/trn_rl_repo")

import numpy as np

N_CORES = 8
P = 128   # partition width
NBLK = 256  # main token-block size
KG_A = 4  # A-table DMA chunks

_prog_cache: dict = {}
last_run_results = None  # BassKernelResults of the most recent dispatch
last_ctx = None          # (nc, in_maps) of the most recent dispatch


def _choose_capacity(nmax: int) -> int:
    """Per-core token capacity: smallest multiple of 64 >= nmax."""
    return ((max(nmax, 1) + 63) // 64) * 64


def _block_list(C: int) -> tuple:
    """Token blocks of NBLK plus one smaller tail (multiple of 64), tail
    LAST so the exposed pipeline drain is small."""
    n, rem = divmod(C, NBLK)
    blocks = [NBLK] * n + ([rem] if rem else [])
    return tuple(blocks)


def _block_kg(bi: int, nb: int) -> int:
    """x DMA chunks per block: fine-grained for block 0 (pipeline fill),
    coarse afterwards (DMA efficiency)."""
    if nb < NBLK:
        return 1
    return 4 if bi == 0 else 1


def _build_program(C: int, H: int, M: int, R: int, O: int):
    """Trace + compile the single SPMD program (shared by all 8 cores)."""
    import concourse.bass as bass
    import concourse.mybir as mybir
    import concourse.tile as tile
    from concourse import bacc

    f32 = mybir.dt.float32
    f16 = mybir.dt.float16
    KT = H // P          # contraction tiles
    KS_A = KT // KG_A
    J = O // 512         # up-projection PSUM tiles per module
    MO = M * O
    blocks = _block_list(C)

    nc = bacc.Bacc("TRN2", target_bir_lowering=False, debug=False,
                   num_devices=N_CORES)

    # xh is flat; block b with KG chunks holds [KG, P, KS, nb] where
    # xh[g, p, k, n] = xT[(g*KS + k)*P + p, t0 + n].
    xh = nc.dram_tensor("xh", [C * H], f16, kind="ExternalInput")
    # wa[g, p, k, j, r] = A_masked[mod(j), (g*KS_A + k)*P + p, r] with
    # module order mod = (0, 2, 1)  (j=0 -> m0, j=1 -> m2, j=2 -> m1).
    wa = nc.dram_tensor("wa", [KG_A, P, KS_A, M, R], f16, kind="ExternalInput")
    # wb[r, m, o] = B[m, r, o] (natural module order, no duplication).
    wb = nc.dram_tensor("wb", [R, M, O], f16, kind="ExternalInput")
    # fp16 output in the FINAL layout [C, M*O]; host widens + row-scatters.
    out = nc.dram_tensor("out", [C, MO], f16, kind="ExternalOutput")

    with tile.TileContext(nc) as tc:
        with (
            tc.tile_pool(name="wgt", bufs=1) as wpool,
            tc.tile_pool(name="yts", bufs=2) as ypool,
            tc.tile_pool(name="ost", bufs=3) as opool,
            tc.tile_pool(name="py", bufs=1, space=bass.MemorySpace.PSUM) as pyp,
            tc.tile_pool(name="pu", bufs=6, space=bass.MemorySpace.PSUM) as pup,
        ):
            wa_t = wpool.tile([P, KT, M, R], f16)
            # wb_t rows 0:R = (m0, m1) tables, rows R:2R col 0 = m2's table.
            wb_t = wpool.tile([2 * R, 2, O], f16)
            for g in range(KG_A):
                nc.scalar.dma_start(wa_t[:, g * KS_A:(g + 1) * KS_A, :, :],
                                    wa[g])

            # One SBUF tile per block (never recycled -> the up-front x DMAs
            # are never gated on compute, so they can't head-of-line block
            # the sync ring).
            xb_tiles = []
            t0 = 0
            for bi, nb in enumerate(blocks):
                xb = wpool.tile([P, KT, nb], f16, tag=f"xb{bi}", name=f"xb{bi}")
                kg = _block_kg(bi, nb)
                ks = KT // kg
                for g in range(kg):
                    off = t0 * H + g * (P * ks * nb)
                    xv = xh[off:off + P * ks * nb].rearrange(
                        "(p k n) -> p k n", p=P, k=ks, n=nb
                    )
                    nc.sync.dma_start(xb[:, g * ks:(g + 1) * ks, :], xv)
                xb_tiles.append(xb)
                t0 += nb
                if bi == 1:
                    # B tables ride between x blocks 1 and 2: early enough
                    # that block 0's up-projection (the first strip work,
                    # ~15us in) is never gated on them, late enough not to
                    # delay block 0/1 down-projections.
                    nc.sync.dma_start(wb_t[0:R, 0:2, :], wb[:, 0:2, :])
                    nc.sync.dma_start(wb_t[R:2 * R, 0, :], wb[:, 2, :])
            if len(blocks) < 2:
                nc.sync.dma_start(wb_t[0:R, 0:2, :], wb[:, 0:2, :])
                nc.sync.dma_start(wb_t[R:2 * R, 0, :], wb[:, 2, :])

            # PE warm-up: junk matmuls keep the HAM activity window busy
            # while the first x chunks stream in, so the PE clock gate is
            # at 8/8 when real work arrives.
            wtile = wpool.tile([P, P], f16)
            nc.gpsimd.memset(wtile[:], 0.0)
            for _ in range(24):
                wu = pup.tile([P, P], f32, tag="ou")
                nc.tensor.matmul(wu[:], wtile[:], wtile[:], start=True, stop=True)

            cp = 0   # PSUM->SBUF copy counter (vector/scalar balancing)

            def _route_copy(dst, src_):
                nonlocal cp
                if cp % 2 == 1:
                    nc.scalar.copy(dst, src_)
                else:
                    nc.vector.tensor_copy(dst, src_)
                cp += 1

            def emit_part(bt0, bnb, byts02, byts1, os_map, s0, which):
                """Up-projection for one 128-row strip: which=0 -> the fused
                m0/m2 pair (concurrent PE row groups), which=1 -> lone m1.
                PSUM tiles are 1024 wide (2 matmuls each, one bank per
                matmul) so each PSUM->SBUF copy moves 2x the elements --
                halves the per-copy fixed cost. Each part ships its own
                column ranges as soon as its copies land."""
                sl = min(P, bnb - s0)
                os_ = os_map[s0]
                r0, r1 = bt0 + s0, bt0 + s0 + sl
                if which == 0:
                    for j in range(J):
                        c0 = j * 512
                        ou0 = pup.tile([P, 512], f32, tag="ou")
                        ou2 = pup.tile([P, 512], f32, tag="ou")
                        nc.tensor.matmul(
                            ou0[:sl, :],
                            byts02[0:R, s0:s0 + sl],
                            wb_t[0:R, 0, c0:c0 + 512],
                            start=True, stop=True,
                        )
                        nc.tensor.matmul(
                            ou2[:sl, :],
                            byts02[R:2 * R, s0:s0 + sl],
                            wb_t[R:2 * R, 0, c0:c0 + 512],
                            start=True, stop=True,
                        )
                        _route_copy(os_[:sl, c0:c0 + 512], ou0[:sl, :])
                        _route_copy(os_[:sl, 2 * O + c0:2 * O + c0 + 512],
                                    ou2[:sl, :])
                    # ship m0+m2 columns now: [sl, 2, O] strided view of the
                    # [C, M*O] output (col ranges 0:O and 2O:3O).
                    nc.sync.dma_start(
                        out[r0:r1].rearrange("t (c o) -> t c o", c=M, o=O)[:, 0:M:2, :],
                        os_[:sl].rearrange("t (c o) -> t c o", c=M, o=O)[:, 0:M:2, :],
                    )
                else:
                    for j in range(J):
                        c0 = j * 512
                        ou1 = pup.tile([P, 512], f32, tag="ou")
                        nc.tensor.matmul(
                            ou1[:sl, :],
                            byts1[0:R, s0:s0 + sl],
                            wb_t[0:R, 1, c0:c0 + 512],
                            start=True, stop=True,
                        )
                        _route_copy(os_[:sl, O + c0:O + c0 + 512], ou1[:sl, :])
                    nc.sync.dma_start(out[r0:r1, O:2 * O], os_[:sl, O:2 * O])

            # Software pipeline with a one-block lag: block b's up-projection
            # parts are emitted BETWEEN block b+1's down-projection chains.
            pend = None  # (t0, nb, yts02, yts1, os_map, parts)
            t0 = 0
            for bi, nb in enumerate(blocks):
                last = bi == len(blocks) - 1
                xb = xb_tiles[bi]
                yts02 = ypool.tile([2 * R, nb], f16, tag="yt02")
                yts1 = ypool.tile([R, nb], f16, tag="yt1")
                os_map = {s0: opool.tile([P, MO], f16, tag="os", name=f"os{bi}")
                          for s0 in range(0, nb, P)}
                parts = [(s0, w) for s0 in range(0, nb, P) for w in (0, 1)]
                ppart = pend[5] if pend else []
                half = (len(ppart) + 1) // 2

                # ---- chain A: modules (m0, m2) fused ----
                y02 = pyp.tile([2 * R, nb], f32, tag="y02")
                for k in range(KT):
                    nc.tensor.matmul(
                        y02[:], wa_t[:, k, 0:2, :], xb[:, k, :],
                        start=(k == 0), stop=(k == KT - 1),
                    )
                nc.vector.tensor_copy(yts02[:], y02[:])
                for s0_, w_ in ppart[:half]:
                    emit_part(pend[0], pend[1], pend[2], pend[3], pend[4],
                              s0_, w_)

                # ---- chain B: module m1 ----
                y1 = pyp.tile([R, nb], f32, tag="y1")
                for k in range(KT):
                    nc.tensor.matmul(
                        y1[:], wa_t[:, k, 2, :], xb[:, k, :],
                        start=(k == 0), stop=(k == KT - 1),
                    )
                nc.vector.tensor_copy(yts1[:], y1[:])
                for s0_, w_ in ppart[half:]:
                    emit_part(pend[0], pend[1], pend[2], pend[3], pend[4],
                              s0_, w_)

                if last:
                    for s0_, w_ in parts:
                        emit_part(t0, nb, yts02, yts1, os_map, s0_, w_)

                pend = (t0, nb, yts02, yts1, os_map, parts)
                t0 += nb

    nc.compile()
    return nc


def _get_program(C: int, H: int, M: int, R: int, O: int):
    key = (C, H, M, R, O)
    if key not in _prog_cache:
        _prog_cache[key] = _build_program(C, H, M, R, O)
    return _prog_cache[key]


def _ensure_profile_hook_module():
    """bass_utils imports antenv.axon_hooks when BASS_TRACE is set; this
    container's antenv package lacks that module. Register a stub returning
    no hook (bass_utils then skips tracing gracefully) unless something
    already provided a real one."""
    import types
    try:
        import antenv.axon_hooks  # noqa: F401
    except ImportError:
        if "antenv.axon_hooks" not in sys.modules:
            mod = types.ModuleType("antenv.axon_hooks")
            mod.get_axon_ntff_profile_hook = lambda: None
            sys.modules["antenv.axon_hooks"] = mod


def kernel(x, lora_a, lora_b, token_adapter_ids, adapter_ranks):
    from concourse.bass_utils import run_bass_kernel_spmd

    _ensure_profile_hook_module()

    x = np.ascontiguousarray(np.asarray(x, dtype=np.float32))
    la = np.array(np.asarray(lora_a), dtype=np.float32, copy=True)  # [M,A,H,R]
    lb = np.ascontiguousarray(np.asarray(lora_b), dtype=np.float32)  # [M,A,R,O]
    ids = np.asarray(token_adapter_ids).astype(np.int64)
    ranks = np.asarray(adapter_ranks).astype(np.int64)

    T, H = x.shape
    M, A, _, R = la.shape
    O = lb.shape[-1]
    assert A <= N_CORES, "one adapter per core"
    assert H % P == 0 and O % 512 == 0

    # Rank masking: zeroing A's columns >= rank_a makes the corresponding
    # intermediate columns exactly 0.0, which is bit-identical to the
    # reference masking the intermediate itself.
    for a in range(A):
        la[:, a, :, int(ranks[a]):] = 0.0

    perms = [np.nonzero(ids == a)[0] for a in range(A)]
    nmax = max(pp.size for pp in perms)
    C = _choose_capacity(nmax)
    blocks = _block_list(C)

    nc = _get_program(C, H, M, R, O)

    KT = H // P
    KS_A = KT // KG_A
    mod_order = (0, 2, 1)
    in_maps = []
    for a in range(N_CORES):
        if a < A:
            perm = perms[a]
            xg = np.zeros((C, H), np.float16)
            xg[:perm.size] = x[perm]  # fp32 -> fp16
            # flat per-block chunked layout [KG, P, KS, nb]
            xh = np.empty(C * H, np.float16)
            t0 = 0
            for bi, nb in enumerate(blocks):
                kg = _block_kg(bi, nb)
                ks = KT // kg
                seg = xg[t0:t0 + nb]  # [nb, H]
                xh[t0 * H:(t0 + nb) * H] = (
                    seg.reshape(nb, kg, ks, P).transpose(1, 3, 2, 0).reshape(-1)
                )
                t0 += nb
            am = la[:, a][list(mod_order)]  # [M, H, R] in (m0, m2, m1) order
            wa_h = np.ascontiguousarray(
                am.reshape(M, KG_A, KS_A, P, R).transpose(1, 3, 2, 0, 4)
            ).astype(np.float16)
            wb_h = np.ascontiguousarray(
                lb[:, a].transpose(1, 0, 2)
            ).astype(np.float16)  # [R, M, O]
        else:
            xh = np.zeros(C * H, np.float16)
            wa_h = np.zeros((KG_A, P, KS_A, M, R), np.float16)
            wb_h = np.zeros((R, M, O), np.float16)
        in_maps.append({"xh": xh, "wa": wa_h, "wb": wb_h})

    global last_run_results, last_ctx
    last_ctx = (nc, in_maps)
    last_run_results = run_bass_kernel_spmd(nc, in_maps, list(range(N_CORES)))
    res = last_run_results.results

    out_full = np.empty((T, M * O), np.float32)
    for a in range(A):
        perm = perms[a]
        if perm.size == 0:
            continue
        r = res[a]["out"]  # [C, M*O] fp16
        out_full[perm] = r[:perm.size, :]
    return out_full
